# revision 35
# baseline (speedup 1.0000x reference)
"""Trainium2 Bass kernel for a binarized (XNOR-style) ResNet BasicBlock.

Reference semantics (per nn_BasicBlock_37228776522124):
    out = BN2(conv3x3(sign(BN1(conv3x3(sign(x), sign(w1)*a1))), sign(w2)*a2)) + x
with training-mode BN (batch stats over N,H,W) and per-out-channel
weight scale a_l = mean(|w_l|).

Key facts exploited:
  * conv inputs are +-1 (or {2,0}-encoded, see below) -> fp8 DoubleRow
    matmuls accumulate EXACT integers in fp32 PSUM.
  * a and BN fold into one per-channel affine s*z + b applied post-conv.
  * activation blocks use two encodings so fills split across ACT/DVE:
      ib0: acts sign(x) in {+1,-1} (ACT Sign), weights +sign(w)
      ib1: acts 2*[x<thr] in {2,0}  (DVE is_lt*2), weights -sign(w)
    ib1's contribution is -2*sum(sign(w)*b) = true - rowsum(sign w);
    the per-channel rowsum constant is added back during PSUM
    evacuation, so stores/stats are of the TRUE conv output.
  * z1 store only feeds sign(z - thr) with thr ~ mean ~ 0: fp8 at z/16
    is sign-safe; z2 store (z/2, even ints <= 1152) is exact in fp16.
  * Data-parallel over batch (4 images/core, 8 cores). BN stats need
    one AllReduce per conv; conv2 runs ob-outer so its stats AR splits
    in two halves and the first finalize half overlaps conv2's tail.

Self-contained: only needs /opt/trn_rl_repo (the Bass toolchain) + numpy.
"""

import os
import sys

for _p in ("/opt/trn_rl_repo",):
    if os.path.isdir(_p) and _p not in sys.path:
        sys.path.insert(0, _p)

import numpy as np

# Problem shapes (hardcoded per spec)
N_FULL, C, H, W = 32, 256, 56, 56
NCORES = 8
NPER = N_FULL // NCORES          # 4 images per core
SP = H * W                       # 3136
HP = H + 2                       # 58 (zero-padded)
SPP = HP * HP                    # 3364
NIB = C // 128                   # 2 input-channel blocks
NOB = C // 128                   # 2 output-channel blocks
NTAP = 9
RB = 7                           # row-blocks of 8 rows
RBW = 8 * W                      # 448 valid outputs per row-block
NMOV = 8 * HP                    # 464 moving columns
RBQ = NMOV + 2                   # 466 f32 <= one psum bank
EPS = 1e-5
KELEM = C * NTAP                 # 2304 weight elems per out channel
ABW = 3376                       # abuf block pitch (16B-aligned for DR pairs)
GB = 2                           # grid base offset inside each block

_nc_cache = {}


def build_nc(num_devices=NCORES):
    import concourse.bacc as bacc
    import concourse.tile as tile
    import concourse.mybir as mybir
    from concourse.bass import broadcast_tensor_aps
    from concourse.masks import make_identity

    F32 = mybir.dt.float32
    F16 = mybir.dt.float16
    BF16 = mybir.dt.bfloat16
    A8 = mybir.dt.float8e4
    ALU = mybir.AluOpType
    ACTF = mybir.ActivationFunctionType
    AX = mybir.AxisListType
    PM = mybir.MatmulPerfMode

    nc = bacc.Bacc(
        "TRN2", target_bir_lowering=False, debug=False,
        num_devices=num_devices,
    )

    x_t = nc.dram_tensor("x", [NPER, C, H, W], F32, kind="ExternalInput")
    w_t = [
        nc.dram_tensor("w1", [C, C, 3, 3], F32, kind="ExternalInput"),
        nc.dram_tensor("w2", [C, C, 3, 3], F32, kind="ExternalInput"),
    ]
    g_t = [
        nc.dram_tensor("gamma1", [C], F32, kind="ExternalInput"),
        nc.dram_tensor("gamma2", [C], F32, kind="ExternalInput"),
    ]
    b_t = [
        nc.dram_tensor("beta1", [C], F32, kind="ExternalInput"),
        nc.dram_tensor("beta2", [C], F32, kind="ExternalInput"),
    ]
    out_t = nc.dram_tensor("out", [NPER, C, H, W], F32, kind="ExternalOutput")

    x_ap = x_t.ap().rearrange("n c h w -> n c (h w)")      # [4, 256, 3136]
    out_ap = out_t.ap().rearrange("n c h w -> n c (h w)")
    rgroups = [list(range(num_devices))]
    M_TOTAL = float(num_devices * NPER * SP)
    zscale = [1.0 / 16.0, 0.5]
    # debug bisection: C1 (conv1+AR1 only) < FULL
    phase_lim = {"C1": 1, "FULL": 9}[os.environ.get("KERNEL_PHASES", "FULL")]

    with tile.TileContext(nc) as tc:
        with (
            tc.tile_pool(name="consts", bufs=1) as pc,
            tc.tile_pool(name="dbl", bufs=2) as pd,
            tc.tile_pool(name="psum", bufs=8, space="PSUM") as pp,
            tc.tile_pool(name="dram", bufs=1, space="DRAM") as pdram,
        ):
            identb = pc.tile([128, 128], BF16, name="identb", tag="identb")
            make_identity(nc, identb[:])
            epsap = pc.tile([128, 1], F32, name="epsap", tag="epsap")
            nc.vector.memset(epsap[:], EPS)

            # persistent z stores: z1/16 fp8, z2/2 fp16 (true values)
            zstore = [
                pc.tile([128, NPER * NOB * SP], A8 if l == 0 else F16,
                        name=f"z{l}", tag=f"z{l}")
                for l in range(2)
            ]
            wsign = [
                pc.tile([128, NTAP * NOB * NIB * 128], A8,
                        name=f"ws{l}", tag=f"ws{l}")
                for l in range(2)
            ]
            alphar = [pc.tile([128, NOB], F32, name=f"al{l}", tag=f"al{l}")
                      for l in range(2)]
            # rowsum(sign w) over ib1 taps, per out channel, times zscale
            rs16 = [pc.tile([128, NOB], F32, name=f"rs{l}", tag=f"rs{l}")
                    for l in range(2)]
            sumc = [pc.tile([128, NOB * 28], F32, name=f"sc{l}", tag=f"sc{l}")
                    for l in range(2)]
            sqc = [pc.tile([128, NOB * 28], F32, name=f"qc{l}", tag=f"qc{l}")
                   for l in range(2)]
            # conv1: one [128,4] stats blob; conv2: per-ob [128,2]
            statloc1 = pc.tile([128, 4], F32, name="sl1", tag="sl1")
            statg1 = pc.tile([128, 4], F32, name="sg1", tag="sg1")
            statloc2 = [pc.tile([128, 2], F32, name=f"sl2{ob}", tag=f"sl2{ob}")
                        for ob in range(NOB)]
            statg2 = [pc.tile([128, 2], F32, name=f"sg2{ob}", tag=f"sg2{ob}")
                      for ob in range(NOB)]
            gb = [pc.tile([128, 2 * NOB], F32, name=f"gb{l}", tag=f"gb{l}")
                  for l in range(2)]
            coef = [pc.tile([128, 2 * NOB], F32, name=f"cf{l}", tag=f"cf{l}")
                    for l in range(2)]
            tau1 = pc.tile([128, NOB], F32, name="tau1", tag="tau1")
            btmp = [pc.tile([128, 14], F32, name=f"bt{l}", tag=f"bt{l}")
                    for l in range(2)]
            # 4 persistent activation buffers (shared by conv1 + conv2)
            abufs = [pc.tile([128, NIB * ABW], A8, name=f"ab{n}", tag=f"ab{n}")
                     for n in range(NPER)]

            # borders/margins set once; fills only rewrite interiors.
            # ib0 (+-1 encoding): zero-pad = 0 contribution.
            # ib1 ({2,0} encoding, weights -sign(w)): pad must be 1.0 so a
            # padded tap contributes -sign(w), matching the uniform
            # +rowsum(sign w) correction applied at evacuation (a zero pad
            # would make the correction wrong at image borders).
            def border_memsets(n, eng):
                for ib in range(NIB):
                    pv = 0.0 if ib == 0 else 1.0
                    a58 = abufs[n][:, ib * ABW + GB:ib * ABW + GB + SPP
                                   ].rearrange("p (h w) -> p h w", w=HP)
                    eng.memset(a58[:, 0:1, :], pv)
                    eng.memset(a58[:, HP - 1:HP, :], pv)
                    eng.memset(a58[:, :, 0:1], pv)
                    eng.memset(a58[:, :, HP - 1:HP], pv)
                    eng.memset(abufs[n][:, ib * ABW:ib * ABW + GB], pv)
                    eng.memset(
                        abufs[n][:, ib * ABW + GB + SPP:(ib + 1) * ABW], pv)

            border_memsets(0, nc.vector)
            border_memsets(1, nc.vector)

            # dummy AllReduce at kernel start: absorbs the first-collective
            # setup (~70us) concurrently with conv1 so real ARs are fast
            ard_i = pdram.tile([128, 1], F32, name="ard_i", tag="ard_i")
            ard_o = pdram.tile([128, 1], F32, name="ard_o", tag="ard_o")
            nc.gpsimd.dma_start(ard_i[:], g_t[0].ap()[0:128])
            nc.gpsimd.collective_compute(
                "AllReduce", ALU.add, replica_groups=rgroups,
                ins=[ard_i.opt()], outs=[ard_o.opt()],
            )

            # ---------------- weight prep ----------------
            # wraw [o, (i,t)] -> wsA = sign(w) bf16 -> PE transpose per tap
            # block -> DVE copy (+1 pairs) / negate (-1 pairs) into fp8
            def weight_prep(l, ob, dma_eng, upto=None, start=0):
                """Emit prep for conv l, out-block ob, tap-blocks
                [start:upto). Returns the (wraw, wsA) tiles on first call."""
                wd = w_t[l].ap().rearrange("o i h w -> o (i h w)")
                wraw = pd.tile([128, KELEM], F32, name="wraw", tag="xfin",
                               bufs=4)
                wsA = pd.tile([128, KELEM], BF16, name="wsA", tag="wsA",
                              bufs=2)
                dma_eng.dma_start(wraw[:], wd[ob * 128:(ob + 1) * 128, :])
                nc.vector.tensor_reduce(
                    out=alphar[l][:, ob:ob + 1], in_=wraw[:],
                    axis=AX.X, op=ALU.add, apply_absolute_value=True,
                )
                # all-channel sign in one ACT op
                nc.scalar.activation(out=wsA[:], in_=wraw[:], func=ACTF.Sign)
                # rowsum over ib1 half (i in [128,256) <-> free [1152:2304))
                rstmp = btmp[l][:, 12 + (ob % 2):13 + (ob % 2)]
                nc.vector.tensor_reduce(
                    out=rstmp, in_=wsA[:, KELEM // 2:KELEM],
                    axis=AX.X, op=ALU.add,
                )
                nc.vector.tensor_scalar_mul(
                    rs16[l][:, ob:ob + 1], rstmp, zscale[l])
                wtap = wsA[:].rearrange("p (i t) -> p t i", t=NTAP)
                emit_tap_blocks(l, ob, wtap)

            def emit_tap_blocks(l, ob, wtap, taps=None):
                for t in (taps if taps is not None else range(NTAP)):
                    for ib in range(NIB):
                        kidx = (ob * NTAP + t) * 2 + ib
                        psT = pp.tile([128, 128], BF16, name="cpt", tag="cps")
                        psTb = psT[:, 0:128]
                        nc.tensor.transpose(
                            psTb,
                            wtap[:, t, ib * 128:(ib + 1) * 128],
                            identb[:],
                        )
                        dst = wsign[l][:, kidx * 128:(kidx + 1) * 128]
                        if ib == 0:
                            nc.vector.tensor_copy(dst, psTb)
                        else:
                            nc.vector.tensor_scalar_mul(dst, psTb, -1.0)

            def load_gb(l):
                for ob in range(NOB):
                    nc.gpsimd.dma_start(
                        gb[l][:, ob:ob + 1],
                        g_t[l].ap()[ob * 128:(ob + 1) * 128])
                    nc.gpsimd.dma_start(
                        gb[l][:, NOB + ob:NOB + ob + 1],
                        b_t[l].ap()[ob * 128:(ob + 1) * 128])

            # ---------------- conv fills ----------------
            def fill1(n):
                """acts for conv1: ib0 = sign(x) on ACT; ib1 = 2*[x<0] on DVE."""
                for ib in range(NIB):
                    xin = pd.tile([128, SP], F32, name="xin", tag="xin",
                                  bufs=3)
                    (nc.gpsimd if ib == 0 else nc.scalar).dma_start(
                        xin[:], x_ap[n, ib * 128:(ib + 1) * 128, :])
                    a58 = abufs[n][:, ib * ABW + GB:ib * ABW + GB + SPP
                                   ].rearrange("p (h w) -> p h w", w=HP)
                    xv = xin[:].rearrange("p (h w) -> p h w", w=W)
                    if ib == 0:
                        nc.scalar.activation(
                            out=a58[:, 1:H + 1, 1:W + 1], in_=xv,
                            func=ACTF.Sign)
                    else:
                        nc.vector.tensor_scalar(
                            out=a58[:, 1:H + 1, 1:W + 1], in0=xv,
                            scalar1=0.0, scalar2=2.0,
                            op0=ALU.is_lt, op1=ALU.mult)

            def fill2(n):
                """acts for conv2 from z1 store: ib0 = sign(s*z+b) on ACT;
                ib1 = 2*[z' < tau'] on DVE."""
                for ib in range(NIB):
                    a58 = abufs[n][:, ib * ABW + GB:ib * ABW + GB + SPP
                                   ].rearrange("p (h w) -> p h w", w=HP)
                    zv = zstore[0][:, (n * NOB + ib) * SP:
                                   (n * NOB + ib + 1) * SP].rearrange(
                        "p (h w) -> p h w", w=W)
                    if ib == 0:
                        nc.scalar.activation(
                            out=a58[:, 1:H + 1, 1:W + 1], in_=zv,
                            func=ACTF.Sign,
                            scale=coef[0][:, ib:ib + 1],
                            bias=coef[0][:, NOB + ib:NOB + ib + 1])
                    else:
                        nc.vector.tensor_scalar(
                            out=a58[:, 1:H + 1, 1:W + 1], in0=zv,
                            scalar1=tau1[:, ib:ib + 1], scalar2=2.0,
                            op0=ALU.is_lt, op1=ALU.mult)

            # ---------------- one matmul+evac group ----------------
            def mm_group(l, n, ob):
                abuf = abufs[n]
                ab3 = abuf[:].rearrange("p (two s) -> p two s", two=NIB)
                ps = [pp.tile([128, RBQ], F32, name="cps", tag="cps")
                      for _ in range(RB)]
                for t in range(NTAP):
                    th, tw = t // 3, t % 3
                    base = (ob * NTAP + t) * 2 * 128
                    lhsT = wsign[l][:, base:base + 256].rearrange(
                        "p (two m) -> p two m", two=2)
                    for rb in range(RB):
                        r0 = (rb * 8 + th) * HP
                        if t == 0:
                            rhs = ab3[:, :, r0:r0 + RBQ]
                            outap = ps[rb][:, 0:RBQ]
                        else:
                            rhs = ab3[:, :, GB + r0:GB + r0 + NMOV]
                            outap = ps[rb][:, 2 - tw:2 - tw + NMOV]
                        nc.tensor.matmul(
                            outap, lhsT, rhs,
                            start=(t == 0), stop=(t == NTAP - 1),
                            perf_mode=PM.DoubleRow,
                        )
                zs = zstore[l]
                for rb in range(RB):
                    col = n * RB + rb
                    zsl = zs[:, ((n * NOB + ob) * SP + rb * RBW):
                             ((n * NOB + ob) * SP + (rb + 1) * RBW)
                             ].rearrange("p (h w) -> p h w", w=W)
                    qv = ps[rb][:, 2:2 + NMOV].rearrange(
                        "p (h w) -> p h w", w=HP)[:, :, 0:W]
                    # store true z*zscale = psum*zscale + rowsum*zscale
                    # (ib1's {2,0}-encoding is short by rowsum(sign w));
                    # accum_out gives sum(true z * zscale) for BN stats
                    rsb = rs16[l][:, ob:ob + 1].rearrange(
                        "p (a b) -> p a b", a=1)
                    _, rs_bb = broadcast_tensor_aps(qv, rsb)
                    nc.vector.scalar_tensor_tensor(
                        out=zsl, in0=qv, scalar=zscale[l], in1=rs_bb,
                        op0=ALU.mult, op1=ALU.add,
                        accum_out=sumc[l][:, ob * 28 + col:ob * 28 + col + 1],
                    )
                    # sum of (psum z)^2 via in-place Square on the dead
                    # bank; the rowsum shift is corrected in the fold
                    nc.scalar.activation(
                        out=qv, in_=qv, func=ACTF.Square,
                        accum_out=sqc[l][:, ob * 28 + col:ob * 28 + col + 1],
                    )

            # ---------------- BN fold ----------------
            def fold(l, ob, statg_ap, scol, qcol, make_tau):
                """statg_ap: [128, >=2] with sum(z_true*zscale) at scol and
                sum(z_shift^2) at qcol. z_shift = z_true - rs (rs = rowsum).
                E[z^2] = E[z_shift^2] + 2*rs*E[z] - rs^2."""
                tmp = btmp[l]
                o6 = (ob % 2) * 6
                mean = tmp[:, 0 + o6:1 + o6]
                e2 = tmp[:, 1 + o6:2 + o6]
                var = tmp[:, 2 + o6:3 + o6]
                alp = tmp[:, 3 + o6:4 + o6]
                tt = tmp[:, 4 + o6:5 + o6]
                std = tmp[:, 5 + o6:6 + o6]
                rs = tmp[:, 13:14]
                # rs (true rowsum) = rs16 / zscale
                nc.vector.tensor_scalar_mul(
                    rs, rs16[l][:, ob:ob + 1], 1.0 / zscale[l])
                nc.vector.tensor_scalar_mul(
                    mean, statg_ap[:, scol:scol + 1],
                    1.0 / (zscale[l] * M_TOTAL))
                nc.vector.tensor_scalar_mul(
                    e2, statg_ap[:, qcol:qcol + 1], 1.0 / M_TOTAL)
                # e2 += 2*rs*mean - rs^2  -> E[z_true^2]
                nc.vector.tensor_mul(tt, rs, mean)
                nc.vector.tensor_scalar_mul(tt, tt, 2.0)
                nc.vector.tensor_add(e2, e2, tt)
                nc.vector.tensor_mul(tt, rs, rs)
                nc.vector.tensor_sub(e2, e2, tt)
                nc.vector.tensor_mul(var, mean, mean)
                nc.vector.tensor_sub(var, e2, var)
                nc.vector.tensor_scalar_mul(
                    alp, alphar[l][:, ob:ob + 1], 1.0 / KELEM)
                nc.vector.tensor_mul(tt, alp, alp)
                nc.vector.tensor_mul(tt, tt, var)
                nc.scalar.activation(std, tt, ACTF.Sqrt, bias=epsap[:])
                nc.vector.reciprocal(tt, std)
                nc.vector.tensor_mul(tt, tt, alp)                # alpha*inv
                nc.vector.tensor_mul(tt, tt, gb[l][:, ob:ob + 1])  # *gamma
                nc.vector.tensor_scalar_mul(
                    coef[l][:, ob:ob + 1], tt, 1.0 / zscale[l])  # s'
                nc.vector.tensor_mul(tt, tt, mean)
                nc.vector.tensor_sub(
                    coef[l][:, NOB + ob:NOB + ob + 1],
                    gb[l][:, NOB + ob:NOB + ob + 1], tt)         # b
                if make_tau:
                    # tau' = -b / s'  (s' > 0 since gamma=1)
                    nc.vector.reciprocal(tt, coef[l][:, ob:ob + 1])
                    nc.vector.tensor_mul(
                        tt, tt, coef[l][:, NOB + ob:NOB + ob + 1])
                    nc.vector.tensor_scalar_mul(
                        tau1[:, ob:ob + 1], tt, -1.0)

            def fold1_wide():
                """BN1 fold for both obs at once ([128,2]-wide ops)."""
                l = 0
                tmp = btmp[0]
                mean, e2 = tmp[:, 0:2], tmp[:, 2:4]
                var, alp = tmp[:, 4:6], tmp[:, 6:8]
                tt, std = tmp[:, 8:10], tmp[:, 10:12]
                rs = tmp[:, 12:14]
                nc.vector.tensor_scalar_mul(rs, rs16[l][:, 0:2],
                                            1.0 / zscale[l])
                nc.vector.tensor_scalar_mul(
                    mean, statg1[:, 0:2], 1.0 / (zscale[l] * M_TOTAL))
                nc.vector.tensor_scalar_mul(e2, statg1[:, 2:4], 1.0 / M_TOTAL)
                nc.vector.tensor_mul(tt, rs, mean)
                nc.vector.tensor_scalar_mul(tt, tt, 2.0)
                nc.vector.tensor_add(e2, e2, tt)
                nc.vector.tensor_mul(tt, rs, rs)
                nc.vector.tensor_sub(e2, e2, tt)
                nc.vector.tensor_mul(var, mean, mean)
                nc.vector.tensor_sub(var, e2, var)
                nc.vector.tensor_scalar_mul(alp, alphar[l][:, 0:2],
                                            1.0 / KELEM)
                nc.vector.tensor_mul(tt, alp, alp)
                nc.vector.tensor_mul(tt, tt, var)
                nc.scalar.activation(std, tt, ACTF.Sqrt, bias=epsap[:])
                nc.vector.reciprocal(tt, std)
                nc.vector.tensor_mul(tt, tt, alp)
                nc.vector.tensor_mul(tt, tt, gb[l][:, 0:2])
                nc.vector.tensor_scalar_mul(
                    coef[l][:, 0:2], tt, 1.0 / zscale[l])
                nc.vector.tensor_mul(tt, tt, mean)
                nc.vector.tensor_sub(coef[l][:, 2:4], gb[l][:, 2:4], tt)
                # tau' = -b / s' for both obs
                nc.vector.reciprocal(tt, coef[l][:, 0:2])
                nc.vector.tensor_mul(tt, tt, coef[l][:, 2:4])
                nc.vector.tensor_scalar_mul(tau1[:, 0:2], tt, -1.0)

            # ================ program ================
            # conv1 weight prep interleaved with the first image fills;
            # DMAs spread over all three rings for parallel startup
            weight_prep(0, 0, nc.sync)
            weight_prep(0, 1, nc.scalar)
            fill1(0)
            fill1(1)
            mm_group(0, 0, 0)
            mm_group(0, 0, 1)
            fill1(2)
            mm_group(0, 1, 0)
            mm_group(0, 1, 1)
            fill1(3)
            load_gb(0)
            load_gb(1)
            border_memsets(2, nc.gpsimd)
            border_memsets(3, nc.gpsimd)
            mm_group(0, 2, 0)
            mm_group(0, 2, 1)
            mm_group(0, 3, 0)
            mm_group(0, 3, 1)

            # local conv1 stats -> [sum_ob0, sum_ob1, sq_ob0, sq_ob1]
            for ob in range(NOB):
                nc.vector.tensor_reduce(
                    out=statloc1[:, ob:ob + 1],
                    in_=sumc[0][:, ob * 28:(ob + 1) * 28],
                    axis=AX.X, op=ALU.add)
                nc.vector.tensor_reduce(
                    out=statloc1[:, NOB + ob:NOB + ob + 1],
                    in_=sqc[0][:, ob * 28:(ob + 1) * 28],
                    axis=AX.X, op=ALU.add)
            arin1 = pdram.tile([128, 4], F32, name="ari1", tag="ari1")
            arout1 = pdram.tile([128, 4], F32, name="aro1", tag="aro1")
            nc.scalar.dma_start(arin1[:], statloc1[:])
            nc.gpsimd.collective_compute(
                "AllReduce", ALU.add, replica_groups=rgroups,
                ins=[arin1.opt()], outs=[arout1.opt()])
            nc.scalar.dma_start(statg1[:], arout1[:])

            # conv2 weight prep fills the AR1 window; tile_wait_until places
            # its PE transposes after conv1's matmuls in the PE queue
            with tc.tile_wait_until(0.115):
                weight_prep(1, 0, nc.sync)
                weight_prep(1, 1, nc.sync)

            if phase_lim < 9:
                dbg = pd.tile([128, SP], F32, name="dbg", tag="xin", bufs=3)
                nc.vector.tensor_copy(dbg[:], zstore[0][:, 0:SP])
                nc.sync.dma_start(out_ap[0, 0:128, :], dbg[:])

            if phase_lim >= 9:
                # BN1 fold + thresholds (both obs vectorized)
                fold1_wide()

                # x reload for the residual add: 8 blocks, ob-major to
                # match finalize consumption order. tile_wait_until keeps
                # these behind conv1's traffic in the sync DMA FIFO.
                xf = []
                with tc.tile_wait_until(0.16):
                    for k in range(2 * NPER):
                        ob, n = k // NPER, k % NPER
                        t = pd.tile([128, SP], F32, name="xfin", tag="xfin",
                                    bufs=4)
                        nc.sync.dma_start(
                            t[:], x_ap[n, ob * 128:(ob + 1) * 128, :])
                        xf.append(t)

                # ---------- conv2, ob-outer ----------
                fill2(0)
                fill2(1)
                mm_group(1, 0, 0)
                fill2(2)
                mm_group(1, 1, 0)
                fill2(3)
                mm_group(1, 2, 0)
                mm_group(1, 3, 0)
                # first-half stats + AR2a
                nc.vector.tensor_reduce(
                    out=statloc2[0][:, 0:1], in_=sumc[1][:, 0:28],
                    axis=AX.X, op=ALU.add)
                nc.vector.tensor_reduce(
                    out=statloc2[0][:, 1:2], in_=sqc[1][:, 0:28],
                    axis=AX.X, op=ALU.add)
                arin2a = pdram.tile([128, 2], F32, name="ari2a", tag="ari2a")
                arout2a = pdram.tile([128, 2], F32, name="aro2a", tag="aro2a")
                nc.gpsimd.dma_start(arin2a[:], statloc2[0][:])
                nc.gpsimd.collective_compute(
                    "AllReduce", ALU.add, replica_groups=rgroups,
                    ins=[arin2a.opt()], outs=[arout2a.opt()])
                nc.sync.dma_start(statg2[0][:], arout2a[:])

                def fin(ob, n):
                    k = ob * NPER + n
                    t1 = pd.tile([128, SP], F32, name="t1", tag="xin",
                                 bufs=3)
                    nc.scalar.activation(
                        out=t1[:],
                        in_=zstore[1][:, (n * NOB + ob) * SP:
                                      (n * NOB + ob + 1) * SP],
                        func=ACTF.Identity,
                        scale=coef[1][:, ob:ob + 1],
                        bias=coef[1][:, NOB + ob:NOB + ob + 1])
                    nc.vector.tensor_add(xf[k][:], t1[:], xf[k][:])
                    (nc.gpsimd if n % 2 == 0 else nc.sync).dma_start(
                        out_ap[n, ob * 128:(ob + 1) * 128, :], xf[k][:])

                mm_group(1, 0, 1)
                mm_group(1, 1, 1)
                # ob0 finalize rides under ob1's matmuls (AR2a has landed)
                fold(1, 0, statg2[0], 0, 1, False)
                fin(0, 0)
                fin(0, 1)
                mm_group(1, 2, 1)
                fin(0, 2)
                fin(0, 3)
                mm_group(1, 3, 1)

                nc.vector.tensor_reduce(
                    out=statloc2[1][:, 0:1], in_=sumc[1][:, 28:56],
                    axis=AX.X, op=ALU.add)
                nc.vector.tensor_reduce(
                    out=statloc2[1][:, 1:2], in_=sqc[1][:, 28:56],
                    axis=AX.X, op=ALU.add)
                arin2b = pdram.tile([128, 2], F32, name="ari2b", tag="ari2b")
                arout2b = pdram.tile([128, 2], F32, name="aro2b", tag="aro2b")
                nc.gpsimd.dma_start(arin2b[:], statloc2[1][:])
                nc.gpsimd.collective_compute(
                    "AllReduce", ALU.add, replica_groups=rgroups,
                    ins=[arin2b.opt()], outs=[arout2b.opt()])
                nc.sync.dma_start(statg2[1][:], arout2b[:])

                fold(1, 1, statg2[1], 0, 1, False)
                for n in range(NPER):
                    fin(1, n)

            # park the (unused) dummy-AR result so DCE keeps the collective;
            # at the very end of the gpsimd ring so it can't block anything
            nc.gpsimd.dma_start(btmp[0][:, 12:13], ard_o[:])

    nc.compile()
    return nc


def _get_nc(num_devices=NCORES):
    if num_devices not in _nc_cache:
        _nc_cache[num_devices] = build_nc(num_devices)
    return _nc_cache[num_devices]


def kernel(**inputs):
    from concourse.bass_utils import run_bass_kernel_spmd

    nc = _get_nc(NCORES)
    x = np.ascontiguousarray(np.asarray(inputs["x"], dtype=np.float32))
    shared = {
        k: np.ascontiguousarray(np.asarray(inputs[k], dtype=np.float32))
        for k in ("w1", "gamma1", "beta1", "w2", "gamma2", "beta2")
    }
    in_maps = [
        {"x": x[c * NPER:(c + 1) * NPER], **shared} for c in range(NCORES)
    ]
    res = run_bass_kernel_spmd(nc, in_maps, core_ids=list(range(NCORES)))
    out = np.concatenate([r["out"] for r in res.results], axis=0)
    return out.astype(np.float32)
